# revision 35
# baseline (speedup 1.0000x reference)
"""NT-Xent contrastive loss on 8 Trainium2 NeuronCores — symmetric-block v2.

Reference computation (B=4096, D=128, T=0.5):
    z = row-normalize(concat(emb_i, emb_j))           # [8192, 128]
    sim = z @ z.T                                     # [8192, 8192]
    S_r = sum_l exp(sim[r,l]/T),  denom_r = S_r - exp(sim[r,r]/T)
    pos_r = sim[r, r+-B]
    loss = mean_r ( log(denom_r) - pos_r/T )

sim is symmetric, so each [1024,1024] block (i,j) of exp(sim/T) only needs
to be exponentiated once: its row-sums feed rows of group i and its
column-sums feed rows of group j.  The 8x8 block grid has 36 unique blocks
(8 diagonal + 28 off-diagonal); each core computes exactly 4.5 of them:

    core c: blocks (c, c+k mod 8) for k=0..3  (diag + 3 off-diag)
            + half of the gap-4 pair {c, (c+4)%8}:
              cores 0-3 take the two diagonal quadrants of (c, c+4),
              cores 4-7 take the two anti-diagonal quadrants, expressed
              uniformly by feeding them slot 4 rotated by 512 rows.

This HALVES the ScalarE exp work vs the v1 kernel (exp throughput is
1 elem/lane/cycle and was the bottleneck).  Column-sums of exp blocks are
produced on the PE with a ones[128,32] stationary into quarter-partition
PSUM accumulators.  Each core returns raw partial sums (row-sums, col-sums,
positives); the host assembles S_r, subtracts the exp(sim_rr/T)=e^2
diagonal, and takes log/mean in float64.

Per-core inputs: `gather` [5*1024, 128] f32 = row groups
    [G(c), G(c+1), G(c+2), G(c+3), H]  (mod 8), where
    H = G(c+4) for c<4, and H = roll(G(c-4), -4 rows) for c>=4.
The SPMD program is identical on every core; only the data differs.

SBUF row layout: slot row r sits at (partition, tile) = (r // 8, r % 8),
so each slot's DMA is one fully contiguous 512KB DRAM stream (the
(p, t) = (r % 128, r // 128) layout read 512B-scattered at ~50GB/s and
made the prologue the bottleneck).  zT column t*128+q therefore holds row
8q+t; the host maps column positions back to rows.  "Half" of a group for
the gap-4 split is the mod-8 row class {0-3} vs {4-7}, which is why the
c>=4 rotation is roll(-4): it swaps the classes so the two cores of a
gap-4 pair cover complementary class-quadrants of the block.

Engine plan per core (predicted busy): ACT ~45us (24 Exp ACTIVATEs of
N=1536 from PSUM + accum row-sums), PE ~30us (72 sim matmuls N=512 bf16 —
one stationary per m-tile serves all 9 — 56 colsum matmuls, 40 bf16
transposes), DVE ~25us (square/reduce/normalize per slot + PSUM->SBUF
copies), DMA 2.7MB.  Norms use exp(-0.5*ln(s)) to stay in the single
natural_log_exp activation-table set (no 2.7us table swaps).
"""

import math

import numpy as np

import concourse.bass as bass
import concourse.mybir as mybir
import concourse.tile as tile
from concourse import masks
from concourse.bass_utils import run_bass_kernel_spmd

B = 4096
D = 128
NR = 2 * B               # 8192 rows of reps / sim
N_CORES = 8
RPC = NR // N_CORES      # 1024 rows per group
P = 128                  # partitions
NG = 8                   # global row groups of 1024
NS = 5                   # gather slots per core
MT = RPC // P            # 8 row tiles per group
TEMPERATURE = 0.5
INV_T = 1.0 / TEMPERATURE          # 2.0
E2 = math.exp(1.0 / TEMPERATURE)   # exp(sim_rr / T), sim_rr == 1

# main loop geometry: per m-tile, 9 chunks of 512 sim columns
#   chunk 0..1 -> block k=0 (diag), 2..3 -> k=1, 4..5 -> k=2, 6..7 -> k=3,
#   chunk 8    -> half-block (slot4 cols 0:512 for m<4, 512:1024 for m>=4)
# Two passes over m (AP base partitions are limited to {0,32,64}, so only
# 4 colsum chains fit the 2 spare PSUM banks at a time):
#   pass 1: batches j=0 (chunks 0,1,2) and j=1 (chunks 3,4,5) per m;
#           colsum chains b1c0,b1c1,b2c0,b2c1 -> drain to SBUF
#   pass 2: batch j=2 (chunks 6,7,8) per m;
#           colsum chains b3c0,b3c1,half(m<4),half(m>=4)
NCHUNK = 9
NBATCH = 3               # chunks per ACTIVATE batch (N=1536)
# cs slot -> (psum bank, partition band)
SLOT_POS = [(0, 0), (0, 32), (0, 64), (1, 0)]

# output packing: [128, 24 esums | 8 pos | 2*1024 colsum drains] f32
OUT_ES = 0               # esums[p, 3*m+j]
OUT_POS = 24             # pos[p, t]
OUT_CS = 32              # per pass: slot s at (bank,band): col 512*bank+k
OUT_W = 32 + 2 * 1024

_NC = None
TRACE = False            # test.py flips this for profiled runs
_LAST_RESULT = None      # test.py reads exec_time_ns / trace from here

f32 = mybir.dt.float32
bf16 = mybir.dt.bfloat16
AF = mybir.ActivationFunctionType
OP = mybir.AluOpType


def _patched_clear_and_free_semaphores(self, sems):
    """Replacement for Bass.clear_and_free_semaphores: the stock version
    emits a raw-ISA EVENT_SEMAPHORE_RANGE_CLEAR that this toolchain's walrus
    rejects ("ISA wrong length").  Emit BIR-native per-sem `wr-imm 0`
    updates on gpsimd NOPs instead — same semantics (sems reset between
    NEFF executions), supported lowering."""
    if not sems:
        return
    sem_nums = [s.num if hasattr(s, "num") else s for s in sems]
    for n in sem_nums:
        inst = self.gpsimd.nop()
        upd = mybir.SyncUpdate(
            sync_type="semaphore",
            id=n,
            update_mode="sem-wr-imm",
            update_value=0,
            ant_name=f"semclr{n}",
        )
        si = inst.ins.sync_info
        if si is None:
            inst.ins.sync_info = mybir.SyncInfo(on_wait=[], on_update=[upd])
        else:
            si.on_update.append(upd)
    self._state.prepend_free_semaphores(sem_nums)
    for poison_set in self._tile_sem_poison_stack:
        poison_set.update(sem_nums)


def _hoist_excess_waits(nc):
    """This toolchain's walrus (CoreV3GenImpl) allows only ONE sync-wait on
    most compute instruction structs; Tile sometimes attaches two.  Hoist
    all-but-one wait onto same-engine EventSemaphore carriers (2 wait slots
    each) inserted immediately before the instruction — same-engine program
    order makes this semantically identical."""
    n = 0
    for f in nc.m.functions:
        for blk in f.blocks:
            out = []
            for inst in blk.instructions:
                si = inst.sync_info
                tn = type(inst).__name__
                if (
                    si is not None
                    and len(si.on_wait) > 1
                    and tn != "InstEventSemaphore"
                ):
                    waits = list(si.on_wait)
                    keep, extra = waits[-1:], waits[:-1]
                    while extra:
                        grp, extra = extra[:2], extra[2:]
                        es = mybir.InstEventSemaphore(
                            name=f"wcarrier_{n}", ins=[], outs=[]
                        )
                        n += 1
                        es.engine = inst.engine
                        es.sync_info = mybir.SyncInfo(on_wait=list(grp), on_update=[])
                        out.append(es)
                    inst.sync_info = mybir.SyncInfo(
                        on_wait=keep, on_update=list(si.on_update)
                    )
                out.append(inst)
            blk.instructions[:] = out


def _build_nc(hoist: bool = True) -> bass.Bass:
    nc = bass.Bass("TRN2", target_bir_lowering=False, debug=False)
    import types as _types

    nc.clear_and_free_semaphores = _types.MethodType(
        _patched_clear_and_free_semaphores, nc
    )

    gather = nc.dram_tensor("gather", [NS * RPC, D], bf16, kind="ExternalInput")
    out_d = nc.dram_tensor("out", [P, 32], f32, kind="ExternalOutput")
    cs_d = nc.dram_tensor("ocs", [8, 512], f32, kind="ExternalOutput")

    with tile.TileContext(nc) as tc:
        with (
            tc.tile_pool(name="singles", bufs=1) as singles,
            tc.tile_pool(name="scratch", bufs=2) as scratch,
            tc.tile_pool(name="expb", bufs=2) as expb,
            tc.tile_pool(name="psum_ring", bufs=2, space="PSUM") as psum_ring,
            # prologue transpose tiles and the colsum accumulators share one
            # 2-bank pool (their lifetimes don't overlap; Tile serializes)
            tc.tile_pool(name="psum_cs", bufs=1, space="PSUM") as psum_cs,
        ):
            ident = singles.tile([P, P], bf16, tag="ident")
            masks.make_identity(nc, ident[:])
            ones32 = singles.tile([P, 32], bf16, tag="ones32")
            nc.vector.memset(ones32[:], 1.0)

            # persistent SBUF buffers
            ld = [
                singles.tile([P, RPC], bf16, name=f"ld{s}", tag=f"ld{s}")
                for s in range(NS)
            ]
            zn = [
                singles.tile([P, RPC], bf16, name=f"zn{s}", tag=f"zn{s}")
                for s in range(NS)
            ]
            zT = [
                singles.tile([P, RPC], bf16, name=f"zT{s}", tag=f"zT{s}")
                for s in range(NS)
            ]
            ss = singles.tile([P, NS * MT], bf16, tag="ss")
            lns = singles.tile([P, NS * MT], f32, tag="lns")
            inv = singles.tile([P, NS * MT], f32, tag="inv")
            outb = singles.tile([P, OUT_W], f32, tag="outb")

            # slot row r at (partition, tile) = (r//8, r%8): contiguous DMA;
            # two half-slot DMAs each so transfers overlap across DMA
            # engines.  Slots 0-1 issue from the Sync queue; slots 2-4 from
            # the Scalar queue (idle until the first Ln anyway) so issue
            # serialization doesn't delay the early slots.
            gv = gather.ap().rearrange("(s p h t) d -> s p h t d",
                                       s=NS, h=2, t=MT // 2)

            # force the ACT activation-table load ahead of the Scalar-queue
            # DMA issues so the first real Ln isn't pushed behind them
            nc.scalar.activation(lns[:, 0:1], ones32[:, 0:1], AF.Ln)

            for s in range(NS):
                ldv = ld[s][:].rearrange("p (h t d) -> p h t d", h=2, d=D)
                eng = nc.sync if s < 2 else nc.scalar
                eng.dma_start(out=ldv[:, 0], in_=gv[s][:, 0])
                eng.dma_start(out=ldv[:, 1], in_=gv[s][:, 1])

            # PE warm-up: ~6us of dummy transposes releases the HAM clock
            # gate (1.2 -> 2.4 GHz) just as the real transposes start, and
            # keeps it released (gaps stay under the 3.4us idle window)
            wt = psum_ring.tile([P, P], bf16, tag="pg", name="warm")
            for _ in range(40):
                nc.tensor.transpose(wt[:], ident[:], ident[:])

            def prep_sq(s):
                # row sum-of-squares (DVE, bf16 2x)
                ssl = slice(s * MT, (s + 1) * MT)
                scr = scratch.tile([P, RPC], bf16, tag="scr", name=f"scr{s}")
                nc.vector.tensor_mul(scr[:], ld[s][:], ld[s][:])
                # bf16 row-norm^2 costs 0.4% relative error on |x| -- far
                # inside the 2e-2 loss tolerance (z is bf16 anyway)
                with nc.allow_low_precision(reason="bf16 norms, tol 2e-2"):
                    nc.vector.tensor_reduce(
                        ss[:, ssl].rearrange("p (t o) -> p t o", o=1),
                        scr[:].rearrange("p (t d) -> p t d", d=D),
                        axis=mybir.AxisListType.X, op=OP.add,
                    )

            def norm_inv(ssl):
                nc.scalar.activation(lns[:, ssl], ss[:, ssl], AF.Ln)
                nc.scalar.activation(inv[:, ssl], lns[:, ssl], AF.Exp, scale=-0.5)

            def prep_norm(s):
                # normalized bf16 rows: per-tile tensor_scalar (bf16 4x
                # mode, f32 per-partition scalar)
                for t in range(MT):
                    tsl = slice(t * D, (t + 1) * D)
                    c = s * MT + t
                    nc.vector.tensor_scalar_mul(
                        zn[s][:, tsl], ld[s][:, tsl], inv[:, c:c + 1]
                    )

            def prep(s):
                prep_sq(s)
                norm_inv(slice(s * MT, (s + 1) * MT))
                prep_norm(s)

            def tpcp(s):
                # PE transposes + PSUM->SBUF copy for zT[s]
                pt = psum_ring.tile([P, RPC], bf16, tag="pg", name=f"pt{s}")
                for t in range(MT):
                    tsl = slice(t * D, (t + 1) * D)
                    nc.tensor.transpose(pt[:, tsl], zn[s][:, tsl], ident[:])
                nc.vector.tensor_copy(zT[s][:], pt[:])

            def positives():
                # pos[p,t] = zn0[row 8p+t] . zn4[row 8p+t] (cores 0-3 valid)
                pscr = scratch.tile([P, RPC], f32, tag="scr", name="pscr")
                nc.vector.tensor_mul(pscr[:], zn[0][:], zn[4][:])
                nc.vector.tensor_reduce(
                    outb[:, OUT_POS:OUT_POS + MT].rearrange("p (t o) -> p t o", o=1),
                    pscr[:].rearrange("p (t d) -> p t d", d=D),
                    axis=mybir.AxisListType.X, op=OP.add,
                )

            # ---- main loop: two passes over m ----
            def chunk_moving(m, c):
                if c < 8:
                    k, half = divmod(c, 2)
                    return zT[k][:, half * 512:(half + 1) * 512]
                off = 512 * (m // 4)
                return zT[4][:, off:off + 512]

            def cs_slot(m, c):
                # -> (slot, first_m, last_m) or None if no colsum needed
                if c < 2:
                    return None                      # diag block
                if c < 6:
                    return (c - 2, 0, 7)             # pass-1 chains
                if c < 8:
                    return (c - 6, 0, 7)             # pass-2: b3c0, b3c1
                return (2, 0, 3) if m < 4 else (3, 4, 7)

            def emit_sim(m, j):
                pg = psum_ring.tile([P, NBATCH * 512], f32, tag="pg",
                                    name=f"pg{m}_{j}")
                stat = zT[0][:, m * D:(m + 1) * D]
                for i in range(NBATCH):
                    c = j * NBATCH + i
                    nc.tensor.matmul(
                        pg[:, i * 512:(i + 1) * 512], stat, chunk_moving(m, c),
                        start=True, stop=True,
                    )
                return pg

            def emit_exp(m, j, pg, ebuf, eoff):
                nc.scalar.activation(
                    ebuf[:, eoff:eoff + NBATCH * 512], pg[:], AF.Exp,
                    scale=INV_T,
                    accum_out=outb[:, OUT_ES + 3 * m + j:OUT_ES + 3 * m + j + 1],
                )

            def emit_cs(m, j, cst, ebuf, eoff):
                for i in range(NBATCH):
                    c = j * NBATCH + i
                    slot = cs_slot(m, c)
                    if slot is None:
                        continue
                    s, m0, m1 = slot
                    bank, band = SLOT_POS[s]
                    nc.tensor.matmul(
                        cst[band:band + 32, 512 * bank:512 * (bank + 1)],
                        ones32[:, 0:32],
                        ebuf[:, eoff + i * 512:eoff + (i + 1) * 512],
                        start=(m == m0), stop=(m == m1),
                        skip_group_check=True,
                    )

            # Phase schedule (software pipelining):
            #   prep 0-2 -> j0 batches (slots 0,1) -> prep/transpose 2-4
            #   under the exp stream -> j1 batches (slots 1,2) -> pass-2
            #   batches (slots 3,4).  sim(k+1) is emitted before cs(k) so
            #   the PE never waits for ACT's exp of batch k.
            prep(0)
            tpcp(0)
            prep(1)
            tpcp(1)
            prep(2)

            prev = None

            def run_phase(j, cst, hooks=None):
                nonlocal prev
                for m in range(MT):
                    ebuf_m = expb.tile([P, NBATCH * 512], bf16,
                                       tag="eb", name=f"eb{j}_{m}")
                    pg = emit_sim(m, j)
                    if prev is not None:
                        emit_cs(*prev)
                    emit_exp(m, j, pg, ebuf_m, 0)
                    prev = (m, j, cst, ebuf_m, 0)
                    if hooks and m in hooks:
                        hooks[m]()

            def drain(cst, p):
                # drain only the written 32-row bands (bank0: 3, bank1: 1)
                lo = OUT_CS + p * 1024
                nc.vector.tensor_copy(outb[0:96, lo:lo + 512], cst[0:96, 0:512])
                nc.vector.tensor_copy(
                    outb[0:32, lo + 512:lo + 1024], cst[0:32, 512:1024]
                )

            def dma_cs(p):
                # band-gather the 4 written cs slots of pass p (8KB total)
                lo = OUT_CS + p * 1024
                b0 = outb[0:96, lo:lo + 512].rearrange(
                    "(a b) n -> a b n", b=32)[:, 0]
                nc.sync.dma_start(out=cs_d.ap()[4 * p:4 * p + 3], in_=b0)
                nc.sync.dma_start(out=cs_d.ap()[4 * p + 3:4 * p + 4],
                                  in_=outb[0:1, lo + 512:lo + 1024])

            def ln34():
                # slots 3/4 norms: one shared Ln/Exp pair slotted between
                # early j0 exps (squares were emitted in the prologue), and
                # the normalize multiplies on the otherwise-idle DVE; tp2
                # fills the PE slack while ACT runs exp(m0,j0)
                norm_inv(slice(3 * MT, 5 * MT))
                prep_norm(3)
                prep_norm(4)
                tpcp(2)

            prep_sq(3)
            prep_sq(4)
            cst1 = psum_cs.tile([P, 2 * 512], f32, tag="cs", name="cs1")
            run_phase(0, cst1, hooks={
                0: ln34,
                2: lambda: tpcp(3),
                4: lambda: tpcp(4),
                6: positives,
            })
            run_phase(1, cst1)
            emit_cs(*prev)
            prev = None
            drain(cst1, 0)
            dma_cs(0)
            cst2 = psum_cs.tile([P, 2 * 512], f32, tag="cs", name="cs2")
            run_phase(2, cst2)
            emit_cs(*prev)
            drain(cst2, 1)
            dma_cs(1)

            nc.sync.dma_start(out=out_d.ap(), in_=outb[:, 0:32])

    if hoist:
        _hoist_excess_waits(nc)
    return nc


def _get_nc(hoist: bool = True) -> bass.Bass:
    # hoist=False for CoreSim (the wait-carrier EventSemaphores it inserts
    # have no sem updates, which CoreSim's event loop rejects); True for HW.
    global _NC
    if _NC is None or getattr(_NC, "_bass_hoisted", None) != hoist:
        _NC = _build_nc(hoist)
        _NC._bass_hoisted = hoist
    return _NC


def core_gather(reps: np.ndarray, c: int) -> np.ndarray:
    """Host-side shard: 5 row groups for core c (slot 4 rotated on c>=4)."""
    grp = lambda g: reps[(g % NG) * RPC:((g % NG) + 1) * RPC]
    slots = [grp(c + k) for k in range(4)]
    if c < 4:
        slots.append(grp(c + 4))
    else:
        # roll by -4 rows: swaps the mod-8 row classes {0-3} <-> {4-7}
        # (a slot row r sits at (partition r//8, tile r%8) on chip, and the
        # gap-4 block is split by tile class, so this makes the two cores
        # of a pair cover complementary class-quadrants)
        slots.append(np.roll(grp(c + 4), -4, axis=0))
    return np.ascontiguousarray(np.concatenate(slots, axis=0))


# zT column l (transpose-tile t, entry q: l = t*128+q) holds slot row 8q+t
_PI = (8 * (np.arange(RPC) % P) + np.arange(RPC) // P).astype(np.int64)


def assemble(outs) -> float:
    """Host-side unshard: sum partials -> denom -> loss (float64)."""
    S = np.zeros(NR, dtype=np.float64)
    pos = np.zeros(B, dtype=np.float64)
    for c, (o1, ocs) in enumerate(outs):
        o1 = np.asarray(o1, dtype=np.float64)
        ocs = np.asarray(ocs, dtype=np.float64)
        # row-sums: esums[p, 3m+j] for local row 8p+m of group c
        es = o1[:, OUT_ES:OUT_ES + 24].reshape(P, MT, NBATCH).sum(axis=2)
        S[c * RPC:(c + 1) * RPC] += es.reshape(RPC)
        # col-sums: ocs row 4p+s = chain (pass p, slot s); zT col l -> row
        # PI[l].  pass 0: b1c0 b1c1 b2c0 b2c1; pass 1: b3c0 b3c1 half0 half1
        for p in range(2):
            for s in range(4):
                v = ocs[4 * p + s]
                ch = 2 + 4 * p + s               # global chunk id 2..9
                if ch < 8:
                    k, half = divmod(ch, 2)      # block k=1..3, col half
                    g = (c + k) % NG
                    S[g * RPC + _PI[half * 512:(half + 1) * 512]] += v
                else:
                    rho = _PI[(ch - 8) * 512:(ch - 7) * 512]  # slot4 rows
                    g = (c + 4) % NG
                    glob = rho if c < 4 else (rho + 4) % RPC
                    S[g * RPC + glob] += v
        if c < 4:
            pv = o1[:, OUT_POS:OUT_POS + MT]     # pos[p, t] -> row 8p+t
            pos[c * RPC:(c + 1) * RPC] = pv.reshape(RPC)
    denom = S - E2
    loss = (np.log(denom).sum() - 2.0 * INV_T * pos.sum()) / NR
    return float(np.float32(loss))


def kernel(emb_i: np.ndarray, emb_j: np.ndarray) -> np.ndarray:
    global _LAST_RESULT
    import ml_dtypes

    reps = np.ascontiguousarray(
        np.concatenate(
            [np.asarray(emb_i, np.float32), np.asarray(emb_j, np.float32)], axis=0
        )
    )
    assert reps.shape == (NR, D)

    in_maps = [
        {"gather": core_gather(reps, c).astype(ml_dtypes.bfloat16)}
        for c in range(N_CORES)
    ]

    kw = {}
    if TRACE:
        import os
        import tempfile

        kw["tmpdir"] = tempfile.mkdtemp(prefix="trace_", dir=os.getcwd())
    res = run_bass_kernel_spmd(
        _get_nc(), in_maps, list(range(N_CORES)), trace=TRACE, **kw
    )
    _LAST_RESULT = res

    return np.asarray(assemble([(r["out"], r["ocs"]) for r in res.results]))


# revision 37
# speedup vs baseline: 1.0141x; 1.0141x over previous
"""NT-Xent contrastive loss on 8 Trainium2 NeuronCores — symmetric-block v2.

Reference computation (B=4096, D=128, T=0.5):
    z = row-normalize(concat(emb_i, emb_j))           # [8192, 128]
    sim = z @ z.T                                     # [8192, 8192]
    S_r = sum_l exp(sim[r,l]/T),  denom_r = S_r - exp(sim[r,r]/T)
    pos_r = sim[r, r+-B]
    loss = mean_r ( log(denom_r) - pos_r/T )

sim is symmetric, so each [1024,1024] block (i,j) of exp(sim/T) only needs
to be exponentiated once: its row-sums feed rows of group i and its
column-sums feed rows of group j.  The 8x8 block grid has 36 unique blocks
(8 diagonal + 28 off-diagonal); each core computes exactly 4.5 of them:

    core c: blocks (c, c+k mod 8) for k=0..3  (diag + 3 off-diag)
            + half of the gap-4 pair {c, (c+4)%8}:
              cores 0-3 take the two diagonal quadrants of (c, c+4),
              cores 4-7 take the two anti-diagonal quadrants, expressed
              uniformly by feeding them slot 4 rotated by 512 rows.

This HALVES the ScalarE exp work vs the v1 kernel (exp throughput is
1 elem/lane/cycle and was the bottleneck).  Column-sums of exp blocks are
produced on the PE with a ones[128,32] stationary into quarter-partition
PSUM accumulators.  Each core returns raw partial sums (row-sums, col-sums,
positives); the host assembles S_r, subtracts the exp(sim_rr/T)=e^2
diagonal, and takes log/mean in float64.

Per-core inputs: `gather` [5*1024, 128] f32 = row groups
    [G(c), G(c+1), G(c+2), G(c+3), H]  (mod 8), where
    H = G(c+4) for c<4, and H = roll(G(c-4), -4 rows) for c>=4.
The SPMD program is identical on every core; only the data differs.

SBUF row layout: slot row r sits at (partition, tile) = (r // 8, r % 8),
so each slot's DMA is one fully contiguous 512KB DRAM stream (the
(p, t) = (r % 128, r // 128) layout read 512B-scattered at ~50GB/s and
made the prologue the bottleneck).  zT column t*128+q therefore holds row
8q+t; the host maps column positions back to rows.  "Half" of a group for
the gap-4 split is the mod-8 row class {0-3} vs {4-7}, which is why the
c>=4 rotation is roll(-4): it swaps the classes so the two cores of a
gap-4 pair cover complementary class-quadrants of the block.

Engine plan per core (predicted busy): ACT ~45us (24 Exp ACTIVATEs of
N=1536 from PSUM + accum row-sums), PE ~30us (72 sim matmuls N=512 bf16 —
one stationary per m-tile serves all 9 — 56 colsum matmuls, 40 bf16
transposes), DVE ~25us (square/reduce/normalize per slot + PSUM->SBUF
copies), DMA 2.7MB.  Norms use exp(-0.5*ln(s)) to stay in the single
natural_log_exp activation-table set (no 2.7us table swaps).
"""

import math

import numpy as np

import concourse.bass as bass
import concourse.mybir as mybir
import concourse.tile as tile
from concourse import masks
from concourse.bass_utils import run_bass_kernel_spmd

B = 4096
D = 128
NR = 2 * B               # 8192 rows of reps / sim
N_CORES = 8
RPC = NR // N_CORES      # 1024 rows per group
P = 128                  # partitions
NG = 8                   # global row groups of 1024
NS = 5                   # gather slots per core
MT = RPC // P            # 8 row tiles per group
TEMPERATURE = 0.5
INV_T = 1.0 / TEMPERATURE          # 2.0
E2 = math.exp(1.0 / TEMPERATURE)   # exp(sim_rr / T), sim_rr == 1

# main loop geometry: per m-tile, 9 chunks of 512 sim columns
#   chunk 0..1 -> block k=0 (diag), 2..3 -> k=1, 4..5 -> k=2, 6..7 -> k=3,
#   chunk 8    -> half-block (slot4 cols 0:512 for m<4, 512:1024 for m>=4)
# Two passes over m (AP base partitions are limited to {0,32,64}, so only
# 4 colsum chains fit the 2 spare PSUM banks at a time):
#   pass 1: batches j=0 (chunks 0,1,2) and j=1 (chunks 3,4,5) per m;
#           colsum chains b1c0,b1c1,b2c0,b2c1 -> drain to SBUF
#   pass 2: batch j=2 (chunks 6,7,8) per m;
#           colsum chains b3c0,b3c1,half(m<4),half(m>=4)
NCHUNK = 9
NBATCH = 3               # chunks per ACTIVATE batch (N=1536)
# cs slot -> (psum bank, partition band)
SLOT_POS = [(0, 0), (0, 32), (0, 64), (1, 0)]

# output packing: [128, 24 esums | 8 pos | 2*1024 colsum drains] f32
OUT_ES = 0               # esums[p, 3*m+j]
OUT_POS = 24             # pos[p, t]
OUT_CS = 32              # per pass: slot s at (bank,band): col 512*bank+k
OUT_W = 32 + 2 * 1024

_NC = None
TRACE = False            # test.py flips this for profiled runs
_LAST_RESULT = None      # test.py reads exec_time_ns / trace from here

f32 = mybir.dt.float32
bf16 = mybir.dt.bfloat16
AF = mybir.ActivationFunctionType
OP = mybir.AluOpType


def _patched_clear_and_free_semaphores(self, sems):
    """Replacement for Bass.clear_and_free_semaphores: the stock version
    emits a raw-ISA EVENT_SEMAPHORE_RANGE_CLEAR that this toolchain's walrus
    rejects ("ISA wrong length").  Emit BIR-native per-sem `wr-imm 0`
    updates on gpsimd NOPs instead — same semantics (sems reset between
    NEFF executions), supported lowering."""
    if not sems:
        return
    sem_nums = [s.num if hasattr(s, "num") else s for s in sems]
    for n in sem_nums:
        inst = self.gpsimd.nop()
        upd = mybir.SyncUpdate(
            sync_type="semaphore",
            id=n,
            update_mode="sem-wr-imm",
            update_value=0,
            ant_name=f"semclr{n}",
        )
        si = inst.ins.sync_info
        if si is None:
            inst.ins.sync_info = mybir.SyncInfo(on_wait=[], on_update=[upd])
        else:
            si.on_update.append(upd)
    self._state.prepend_free_semaphores(sem_nums)
    for poison_set in self._tile_sem_poison_stack:
        poison_set.update(sem_nums)


def _hoist_excess_waits(nc):
    """This toolchain's walrus (CoreV3GenImpl) allows only ONE sync-wait on
    most compute instruction structs; Tile sometimes attaches two.  Hoist
    all-but-one wait onto same-engine EventSemaphore carriers (2 wait slots
    each) inserted immediately before the instruction — same-engine program
    order makes this semantically identical."""
    n = 0
    for f in nc.m.functions:
        for blk in f.blocks:
            out = []
            for inst in blk.instructions:
                si = inst.sync_info
                tn = type(inst).__name__
                if (
                    si is not None
                    and len(si.on_wait) > 1
                    and tn != "InstEventSemaphore"
                ):
                    waits = list(si.on_wait)
                    keep, extra = waits[-1:], waits[:-1]
                    while extra:
                        grp, extra = extra[:2], extra[2:]
                        es = mybir.InstEventSemaphore(
                            name=f"wcarrier_{n}", ins=[], outs=[]
                        )
                        n += 1
                        es.engine = inst.engine
                        es.sync_info = mybir.SyncInfo(on_wait=list(grp), on_update=[])
                        out.append(es)
                    inst.sync_info = mybir.SyncInfo(
                        on_wait=keep, on_update=list(si.on_update)
                    )
                out.append(inst)
            blk.instructions[:] = out


def _build_nc(hoist: bool = True) -> bass.Bass:
    nc = bass.Bass("TRN2", target_bir_lowering=False, debug=False)
    import types as _types

    nc.clear_and_free_semaphores = _types.MethodType(
        _patched_clear_and_free_semaphores, nc
    )

    gather = nc.dram_tensor("gather", [NS * RPC, D], bf16, kind="ExternalInput")
    out_d = nc.dram_tensor("out", [P, 32], f32, kind="ExternalOutput")
    cs_d = nc.dram_tensor("ocs", [8, 512], f32, kind="ExternalOutput")

    with tile.TileContext(nc) as tc:
        with (
            tc.tile_pool(name="singles", bufs=1) as singles,
            tc.tile_pool(name="scratch", bufs=2) as scratch,
            tc.tile_pool(name="expb", bufs=2) as expb,
            tc.tile_pool(name="psum_ring", bufs=2, space="PSUM") as psum_ring,
            # prologue transpose tiles and the colsum accumulators share one
            # 2-bank pool (their lifetimes don't overlap; Tile serializes)
            tc.tile_pool(name="psum_cs", bufs=1, space="PSUM") as psum_cs,
        ):
            ident = singles.tile([P, P], bf16, tag="ident")
            masks.make_identity(nc, ident[:])
            ones32 = singles.tile([P, 32], bf16, tag="ones32")
            nc.vector.memset(ones32[:], 1.0)

            # persistent SBUF buffers
            ld = [
                singles.tile([P, RPC], bf16, name=f"ld{s}", tag=f"ld{s}")
                for s in range(NS)
            ]
            zn = [
                singles.tile([P, RPC], bf16, name=f"zn{s}", tag=f"zn{s}")
                for s in range(NS)
            ]
            zT = [
                singles.tile([P, RPC], bf16, name=f"zT{s}", tag=f"zT{s}")
                for s in range(NS)
            ]
            ss = singles.tile([P, NS * MT], bf16, tag="ss")
            lns = singles.tile([P, NS * MT], f32, tag="lns")
            inv = singles.tile([P, NS * MT], f32, tag="inv")
            outb = singles.tile([P, OUT_W], f32, tag="outb")

            # slot row r at (partition, tile) = (r//8, r%8): contiguous DMA;
            # two half-slot DMAs each so transfers overlap across DMA
            # engines.  Slots 0-1 issue from the Sync queue; slots 2-4 from
            # the Scalar queue (idle until the first Ln anyway) so issue
            # serialization doesn't delay the early slots.
            gv = gather.ap().rearrange("(s p h t) d -> s p h t d",
                                       s=NS, h=2, t=MT // 2)

            # force the ACT activation-table load ahead of the Scalar-queue
            # DMA issues so the first real Ln isn't pushed behind them
            nc.scalar.activation(lns[:, 0:1], ones32[:, 0:1], AF.Ln)

            for s in range(NS):
                ldv = ld[s][:].rearrange("p (h t d) -> p h t d", h=2, d=D)
                eng = nc.sync if s < 2 else nc.scalar
                eng.dma_start(out=ldv[:, 0], in_=gv[s][:, 0])
                eng.dma_start(out=ldv[:, 1], in_=gv[s][:, 1])

            # PE warm-up: ~6us of dummy transposes releases the HAM clock
            # gate (1.2 -> 2.4 GHz) just as the real transposes start, and
            # keeps it released (gaps stay under the 3.4us idle window)
            wt = psum_ring.tile([P, P], bf16, tag="pg", name="warm")
            for _ in range(64):
                nc.tensor.transpose(wt[:], ident[:], ident[:])

            def prep_sq(s):
                # row sum-of-squares (DVE, bf16 2x)
                ssl = slice(s * MT, (s + 1) * MT)
                scr = scratch.tile([P, RPC], bf16, tag="scr", name=f"scr{s}")
                nc.vector.tensor_mul(scr[:], ld[s][:], ld[s][:])
                # bf16 row-norm^2 costs 0.4% relative error on |x| -- far
                # inside the 2e-2 loss tolerance (z is bf16 anyway)
                with nc.allow_low_precision(reason="bf16 norms, tol 2e-2"):
                    nc.vector.tensor_reduce(
                        ss[:, ssl].rearrange("p (t o) -> p t o", o=1),
                        scr[:].rearrange("p (t d) -> p t d", d=D),
                        axis=mybir.AxisListType.X, op=OP.add,
                    )

            def norm_inv(ssl):
                nc.scalar.activation(lns[:, ssl], ss[:, ssl], AF.Ln)
                nc.scalar.activation(inv[:, ssl], lns[:, ssl], AF.Exp, scale=-0.5)

            def prep_norm(s):
                # normalized bf16 rows: per-tile tensor_scalar (bf16 4x
                # mode, f32 per-partition scalar)
                for t in range(MT):
                    tsl = slice(t * D, (t + 1) * D)
                    c = s * MT + t
                    nc.vector.tensor_scalar_mul(
                        zn[s][:, tsl], ld[s][:, tsl], inv[:, c:c + 1]
                    )

            def prep(s):
                prep_sq(s)
                norm_inv(slice(s * MT, (s + 1) * MT))
                prep_norm(s)

            def tpcp(s):
                # PE transposes + PSUM->SBUF copy for zT[s]
                pt = psum_ring.tile([P, RPC], bf16, tag="pg", name=f"pt{s}")
                for t in range(MT):
                    tsl = slice(t * D, (t + 1) * D)
                    nc.tensor.transpose(pt[:, tsl], zn[s][:, tsl], ident[:])
                nc.vector.tensor_copy(zT[s][:], pt[:])

            def positives():
                # pos[p,t] = zn0[row 8p+t] . zn4[row 8p+t] (cores 0-3 valid)
                pscr = scratch.tile([P, RPC], f32, tag="scr", name="pscr")
                nc.vector.tensor_mul(pscr[:], zn[0][:], zn[4][:])
                nc.vector.tensor_reduce(
                    outb[:, OUT_POS:OUT_POS + MT].rearrange("p (t o) -> p t o", o=1),
                    pscr[:].rearrange("p (t d) -> p t d", d=D),
                    axis=mybir.AxisListType.X, op=OP.add,
                )

            # ---- main loop: two passes over m ----
            def chunk_moving(m, c):
                if c < 8:
                    k, half = divmod(c, 2)
                    return zT[k][:, half * 512:(half + 1) * 512]
                off = 512 * (m // 4)
                return zT[4][:, off:off + 512]

            def cs_slot(m, c):
                # -> (slot, first_m, last_m) or None if no colsum needed
                if c < 2:
                    return None                      # diag block
                if c < 6:
                    return (c - 2, 0, 7)             # pass-1 chains
                if c < 8:
                    return (c - 6, 0, 7)             # pass-2: b3c0, b3c1
                return (2, 0, 3) if m < 4 else (3, 4, 7)

            def emit_sim(m, j):
                pg = psum_ring.tile([P, NBATCH * 512], f32, tag="pg",
                                    name=f"pg{m}_{j}")
                stat = zT[0][:, m * D:(m + 1) * D]
                for i in range(NBATCH):
                    c = j * NBATCH + i
                    nc.tensor.matmul(
                        pg[:, i * 512:(i + 1) * 512], stat, chunk_moving(m, c),
                        start=True, stop=True,
                    )
                return pg

            def emit_exp(m, j, pg, ebuf, eoff):
                nc.scalar.activation(
                    ebuf[:, eoff:eoff + NBATCH * 512], pg[:], AF.Exp,
                    scale=INV_T,
                    accum_out=outb[:, OUT_ES + 3 * m + j:OUT_ES + 3 * m + j + 1],
                )

            def emit_cs(m, j, cst, ebuf, eoff):
                for i in range(NBATCH):
                    c = j * NBATCH + i
                    slot = cs_slot(m, c)
                    if slot is None:
                        continue
                    s, m0, m1 = slot
                    bank, band = SLOT_POS[s]
                    nc.tensor.matmul(
                        cst[band:band + 32, 512 * bank:512 * (bank + 1)],
                        ones32[:, 0:32],
                        ebuf[:, eoff + i * 512:eoff + (i + 1) * 512],
                        start=(m == m0), stop=(m == m1),
                        skip_group_check=True,
                    )

            # Phase schedule (software pipelining):
            #   prep 0-2 -> j0 batches (slots 0,1) -> prep/transpose 2-4
            #   under the exp stream -> j1 batches (slots 1,2) -> pass-2
            #   batches (slots 3,4).  sim(k+1) is emitted before cs(k) so
            #   the PE never waits for ACT's exp of batch k.
            prep(0)
            tpcp(0)
            prep(1)
            tpcp(1)
            prep(2)

            prev = None

            def run_phase(j, cst, hooks=None):
                nonlocal prev
                for m in range(MT):
                    ebuf_m = expb.tile([P, NBATCH * 512], bf16,
                                       tag="eb", name=f"eb{j}_{m}")
                    pg = emit_sim(m, j)
                    if prev is not None:
                        emit_cs(*prev)
                    emit_exp(m, j, pg, ebuf_m, 0)
                    prev = (m, j, cst, ebuf_m, 0)
                    if hooks and m in hooks:
                        hooks[m]()

            def drain(cst, p):
                # drain only the written 32-row bands (bank0: 3, bank1: 1)
                lo = OUT_CS + p * 1024
                nc.vector.tensor_copy(outb[0:96, lo:lo + 512], cst[0:96, 0:512])
                nc.vector.tensor_copy(
                    outb[0:32, lo + 512:lo + 1024], cst[0:32, 512:1024]
                )

            def dma_cs(p):
                # band-gather the 4 written cs slots of pass p (8KB total)
                lo = OUT_CS + p * 1024
                b0 = outb[0:96, lo:lo + 512].rearrange(
                    "(a b) n -> a b n", b=32)[:, 0]
                nc.sync.dma_start(out=cs_d.ap()[4 * p:4 * p + 3], in_=b0)
                nc.sync.dma_start(out=cs_d.ap()[4 * p + 3:4 * p + 4],
                                  in_=outb[0:1, lo + 512:lo + 1024])

            def ln34():
                # slots 3/4 norms: one shared Ln/Exp pair slotted between
                # early j0 exps (squares were emitted in the prologue), and
                # the normalize multiplies on the otherwise-idle DVE; tp2
                # fills the PE slack while ACT runs exp(m0,j0)
                norm_inv(slice(3 * MT, 5 * MT))
                prep_norm(3)
                prep_norm(4)

            prep_sq(3)
            prep_sq(4)
            cst1 = psum_cs.tile([P, 2 * 512], f32, tag="cs", name="cs1")
            run_phase(0, cst1, hooks={
                0: ln34,
                4: lambda: tpcp(2),
                5: lambda: tpcp(3),
                6: lambda: tpcp(4),
                7: positives,
            })
            run_phase(1, cst1)
            emit_cs(*prev)
            prev = None
            drain(cst1, 0)
            dma_cs(0)
            cst2 = psum_cs.tile([P, 2 * 512], f32, tag="cs", name="cs2")
            run_phase(2, cst2)
            emit_cs(*prev)
            drain(cst2, 1)
            dma_cs(1)

            nc.sync.dma_start(out=out_d.ap(), in_=outb[:, 0:32])

    if hoist:
        _hoist_excess_waits(nc)
    return nc


def _get_nc(hoist: bool = True) -> bass.Bass:
    # hoist=False for CoreSim (the wait-carrier EventSemaphores it inserts
    # have no sem updates, which CoreSim's event loop rejects); True for HW.
    global _NC
    if _NC is None or getattr(_NC, "_bass_hoisted", None) != hoist:
        _NC = _build_nc(hoist)
        _NC._bass_hoisted = hoist
    return _NC


def core_gather(reps: np.ndarray, c: int) -> np.ndarray:
    """Host-side shard: 5 row groups for core c (slot 4 rotated on c>=4)."""
    grp = lambda g: reps[(g % NG) * RPC:((g % NG) + 1) * RPC]
    slots = [grp(c + k) for k in range(4)]
    if c < 4:
        slots.append(grp(c + 4))
    else:
        # roll by -4 rows: swaps the mod-8 row classes {0-3} <-> {4-7}
        # (a slot row r sits at (partition r//8, tile r%8) on chip, and the
        # gap-4 block is split by tile class, so this makes the two cores
        # of a pair cover complementary class-quadrants)
        slots.append(np.roll(grp(c + 4), -4, axis=0))
    return np.ascontiguousarray(np.concatenate(slots, axis=0))


# zT column l (transpose-tile t, entry q: l = t*128+q) holds slot row 8q+t
_PI = (8 * (np.arange(RPC) % P) + np.arange(RPC) // P).astype(np.int64)


def assemble(outs) -> float:
    """Host-side unshard: sum partials -> denom -> loss (float64)."""
    S = np.zeros(NR, dtype=np.float64)
    pos = np.zeros(B, dtype=np.float64)
    for c, (o1, ocs) in enumerate(outs):
        o1 = np.asarray(o1, dtype=np.float64)
        ocs = np.asarray(ocs, dtype=np.float64)
        # row-sums: esums[p, 3m+j] for local row 8p+m of group c
        es = o1[:, OUT_ES:OUT_ES + 24].reshape(P, MT, NBATCH).sum(axis=2)
        S[c * RPC:(c + 1) * RPC] += es.reshape(RPC)
        # col-sums: ocs row 4p+s = chain (pass p, slot s); zT col l -> row
        # PI[l].  pass 0: b1c0 b1c1 b2c0 b2c1; pass 1: b3c0 b3c1 half0 half1
        for p in range(2):
            for s in range(4):
                v = ocs[4 * p + s]
                ch = 2 + 4 * p + s               # global chunk id 2..9
                if ch < 8:
                    k, half = divmod(ch, 2)      # block k=1..3, col half
                    g = (c + k) % NG
                    S[g * RPC + _PI[half * 512:(half + 1) * 512]] += v
                else:
                    rho = _PI[(ch - 8) * 512:(ch - 7) * 512]  # slot4 rows
                    g = (c + 4) % NG
                    glob = rho if c < 4 else (rho + 4) % RPC
                    S[g * RPC + glob] += v
        if c < 4:
            pv = o1[:, OUT_POS:OUT_POS + MT]     # pos[p, t] -> row 8p+t
            pos[c * RPC:(c + 1) * RPC] = pv.reshape(RPC)
    denom = S - E2
    loss = (np.log(denom).sum() - 2.0 * INV_T * pos.sum()) / NR
    return float(np.float32(loss))


def kernel(emb_i: np.ndarray, emb_j: np.ndarray) -> np.ndarray:
    global _LAST_RESULT
    import ml_dtypes

    reps = np.ascontiguousarray(
        np.concatenate(
            [np.asarray(emb_i, np.float32), np.asarray(emb_j, np.float32)], axis=0
        )
    )
    assert reps.shape == (NR, D)

    in_maps = [
        {"gather": core_gather(reps, c).astype(ml_dtypes.bfloat16)}
        for c in range(N_CORES)
    ]

    kw = {}
    if TRACE:
        import os
        import tempfile

        kw["tmpdir"] = tempfile.mkdtemp(prefix="trace_", dir=os.getcwd())
    res = run_bass_kernel_spmd(
        _get_nc(), in_maps, list(range(N_CORES)), trace=TRACE, **kw
    )
    _LAST_RESULT = res

    return np.asarray(assemble([(r["out"], r["ocs"]) for r in res.results]))


# revision 40
# speedup vs baseline: 1.0170x; 1.0029x over previous
"""NT-Xent contrastive loss on 8 Trainium2 NeuronCores — symmetric-block v2.

Reference computation (B=4096, D=128, T=0.5):
    z = row-normalize(concat(emb_i, emb_j))           # [8192, 128]
    sim = z @ z.T                                     # [8192, 8192]
    S_r = sum_l exp(sim[r,l]/T),  denom_r = S_r - exp(sim[r,r]/T)
    pos_r = sim[r, r+-B]
    loss = mean_r ( log(denom_r) - pos_r/T )

sim is symmetric, so each [1024,1024] block (i,j) of exp(sim/T) only needs
to be exponentiated once: its row-sums feed rows of group i and its
column-sums feed rows of group j.  The 8x8 block grid has 36 unique blocks
(8 diagonal + 28 off-diagonal); each core computes exactly 4.5 of them:

    core c: blocks (c, c+k mod 8) for k=0..3  (diag + 3 off-diag)
            + half of the gap-4 pair {c, (c+4)%8}:
              cores 0-3 take the two diagonal quadrants of (c, c+4),
              cores 4-7 take the two anti-diagonal quadrants, expressed
              uniformly by feeding them slot 4 rotated by 512 rows.

This HALVES the ScalarE exp work vs the v1 kernel (exp throughput is
1 elem/lane/cycle and was the bottleneck).  Column-sums of exp blocks are
produced on the PE with a ones[128,32] stationary into quarter-partition
PSUM accumulators.  Each core returns raw partial sums (row-sums, col-sums,
positives); the host assembles S_r, subtracts the exp(sim_rr/T)=e^2
diagonal, and takes log/mean in float64.

Per-core inputs: `gather` [5*1024, 128] f32 = row groups
    [G(c), G(c+1), G(c+2), G(c+3), H]  (mod 8), where
    H = G(c+4) for c<4, and H = roll(G(c-4), -4 rows) for c>=4.
The SPMD program is identical on every core; only the data differs.

SBUF row layout: slot row r sits at (partition, tile) = (r // 8, r % 8),
so each slot's DMA is one fully contiguous 512KB DRAM stream (the
(p, t) = (r % 128, r // 128) layout read 512B-scattered at ~50GB/s and
made the prologue the bottleneck).  zT column t*128+q therefore holds row
8q+t; the host maps column positions back to rows.  "Half" of a group for
the gap-4 split is the mod-8 row class {0-3} vs {4-7}, which is why the
c>=4 rotation is roll(-4): it swaps the classes so the two cores of a
gap-4 pair cover complementary class-quadrants of the block.

Engine plan per core (predicted busy): ACT ~45us (24 Exp ACTIVATEs of
N=1536 from PSUM + accum row-sums), PE ~30us (72 sim matmuls N=512 bf16 —
one stationary per m-tile serves all 9 — 56 colsum matmuls, 40 bf16
transposes), DVE ~25us (square/reduce/normalize per slot + PSUM->SBUF
copies), DMA 2.7MB.  Norms use exp(-0.5*ln(s)) to stay in the single
natural_log_exp activation-table set (no 2.7us table swaps).
"""

import math

import numpy as np

import concourse.bass as bass
import concourse.mybir as mybir
import concourse.tile as tile
from concourse import masks
from concourse.bass_utils import run_bass_kernel_spmd

B = 4096
D = 128
NR = 2 * B               # 8192 rows of reps / sim
N_CORES = 8
RPC = NR // N_CORES      # 1024 rows per group
P = 128                  # partitions
NG = 8                   # global row groups of 1024
NS = 5                   # gather slots per core
MT = RPC // P            # 8 row tiles per group
TEMPERATURE = 0.5
INV_T = 1.0 / TEMPERATURE          # 2.0
E2 = math.exp(1.0 / TEMPERATURE)   # exp(sim_rr / T), sim_rr == 1

# main loop geometry: per m-tile, 9 chunks of 512 sim columns
#   chunk 0..1 -> block k=0 (diag), 2..3 -> k=1, 4..5 -> k=2, 6..7 -> k=3,
#   chunk 8    -> half-block (slot4 cols 0:512 for m<4, 512:1024 for m>=4)
# Two passes over m (AP base partitions are limited to {0,32,64}, so only
# 4 colsum chains fit the 2 spare PSUM banks at a time):
#   pass 1: batches j=0 (chunks 0,1,2) and j=1 (chunks 3,4,5) per m;
#           colsum chains b1c0,b1c1,b2c0,b2c1 -> drain to SBUF
#   pass 2: batch j=2 (chunks 6,7,8) per m;
#           colsum chains b3c0,b3c1,half(m<4),half(m>=4)
NCHUNK = 9
NBATCH = 3               # chunks per ACTIVATE batch (N=1536)
# cs slot -> (psum bank, partition band)
SLOT_POS = [(0, 0), (0, 32), (0, 64), (1, 0)]

# output packing: [128, 24 esums | 8 pos | 2*1024 colsum drains] f32
OUT_ES = 0               # esums[p, 3*m+j]
OUT_POS = 24             # pos[p, t]
OUT_CS = 32              # per pass: slot s at (bank,band): col 512*bank+k
OUT_W = 32 + 2 * 1024

_NC = None
TRACE = False            # test.py flips this for profiled runs
_LAST_RESULT = None      # test.py reads exec_time_ns / trace from here

f32 = mybir.dt.float32
bf16 = mybir.dt.bfloat16
AF = mybir.ActivationFunctionType
OP = mybir.AluOpType


def _patched_clear_and_free_semaphores(self, sems):
    """Replacement for Bass.clear_and_free_semaphores: the stock version
    emits a raw-ISA EVENT_SEMAPHORE_RANGE_CLEAR that this toolchain's walrus
    rejects ("ISA wrong length").  Emit BIR-native per-sem `wr-imm 0`
    updates on gpsimd NOPs instead — same semantics (sems reset between
    NEFF executions), supported lowering."""
    if not sems:
        return
    sem_nums = [s.num if hasattr(s, "num") else s for s in sems]
    for n in sem_nums:
        inst = self.gpsimd.nop()
        upd = mybir.SyncUpdate(
            sync_type="semaphore",
            id=n,
            update_mode="sem-wr-imm",
            update_value=0,
            ant_name=f"semclr{n}",
        )
        si = inst.ins.sync_info
        if si is None:
            inst.ins.sync_info = mybir.SyncInfo(on_wait=[], on_update=[upd])
        else:
            si.on_update.append(upd)
    self._state.prepend_free_semaphores(sem_nums)
    for poison_set in self._tile_sem_poison_stack:
        poison_set.update(sem_nums)


def _hoist_excess_waits(nc):
    """This toolchain's walrus (CoreV3GenImpl) allows only ONE sync-wait on
    most compute instruction structs; Tile sometimes attaches two.  Hoist
    all-but-one wait onto same-engine EventSemaphore carriers (2 wait slots
    each) inserted immediately before the instruction — same-engine program
    order makes this semantically identical."""
    n = 0
    for f in nc.m.functions:
        for blk in f.blocks:
            out = []
            for inst in blk.instructions:
                si = inst.sync_info
                tn = type(inst).__name__
                if (
                    si is not None
                    and len(si.on_wait) > 1
                    and tn != "InstEventSemaphore"
                ):
                    waits = list(si.on_wait)
                    keep, extra = waits[-1:], waits[:-1]
                    while extra:
                        grp, extra = extra[:2], extra[2:]
                        es = mybir.InstEventSemaphore(
                            name=f"wcarrier_{n}", ins=[], outs=[]
                        )
                        n += 1
                        es.engine = inst.engine
                        es.sync_info = mybir.SyncInfo(on_wait=list(grp), on_update=[])
                        out.append(es)
                    inst.sync_info = mybir.SyncInfo(
                        on_wait=keep, on_update=list(si.on_update)
                    )
                out.append(inst)
            blk.instructions[:] = out


def _build_nc(hoist: bool = True) -> bass.Bass:
    nc = bass.Bass("TRN2", target_bir_lowering=False, debug=False)
    import types as _types

    nc.clear_and_free_semaphores = _types.MethodType(
        _patched_clear_and_free_semaphores, nc
    )

    gather = nc.dram_tensor("gather", [NS * RPC, D], bf16, kind="ExternalInput")
    out_d = nc.dram_tensor("out", [P, 32], f32, kind="ExternalOutput")
    cs_d = nc.dram_tensor("ocs", [8, 512], f32, kind="ExternalOutput")

    with tile.TileContext(nc) as tc:
        with (
            tc.tile_pool(name="singles", bufs=1) as singles,
            tc.tile_pool(name="scratch", bufs=2) as scratch,
            tc.tile_pool(name="expb", bufs=2) as expb,
            tc.tile_pool(name="psum_ring", bufs=2, space="PSUM") as psum_ring,
            # prologue transpose tiles and the colsum accumulators share one
            # 2-bank pool (their lifetimes don't overlap; Tile serializes)
            tc.tile_pool(name="psum_cs", bufs=1, space="PSUM") as psum_cs,
        ):
            ident = singles.tile([P, P], bf16, tag="ident")
            masks.make_identity(nc, ident[:])
            ones32 = singles.tile([P, 32], bf16, tag="ones32")
            nc.vector.memset(ones32[:], 1.0)

            # persistent SBUF buffers
            ld = [
                singles.tile([P, RPC], bf16, name=f"ld{s}", tag=f"ld{s}")
                for s in range(NS)
            ]
            zn = [
                singles.tile([P, RPC], bf16, name=f"zn{s}", tag=f"zn{s}")
                for s in range(NS)
            ]
            zT = [
                singles.tile([P, RPC], bf16, name=f"zT{s}", tag=f"zT{s}")
                for s in range(NS)
            ]
            ss = singles.tile([P, NS * MT], bf16, tag="ss")
            lns = singles.tile([P, NS * MT], f32, tag="lns")
            inv = singles.tile([P, NS * MT], f32, tag="inv")
            outb = singles.tile([P, OUT_W], f32, tag="outb")

            # slot row r at (partition, tile) = (r//8, r%8): contiguous DMA;
            # two half-slot DMAs each so transfers overlap across DMA
            # engines.  Slots 0-1 issue from the Sync queue; slots 2-4 from
            # the Scalar queue (idle until the first Ln anyway) so issue
            # serialization doesn't delay the early slots.
            gv = gather.ap().rearrange("(s p h t) d -> s p h t d",
                                       s=NS, h=2, t=MT // 2)

            # force the ACT activation-table load ahead of the Scalar-queue
            # DMA issues so the first real Ln isn't pushed behind them
            nc.scalar.activation(lns[:, 0:1], ones32[:, 0:1], AF.Ln)

            for s in range(NS):
                ldv = ld[s][:].rearrange("p (h t d) -> p h t d", h=2, d=D)
                eng = nc.sync if s < 2 else nc.scalar
                eng.dma_start(out=ldv[:, 0], in_=gv[s][:, 0])
                eng.dma_start(out=ldv[:, 1], in_=gv[s][:, 1])

            # PE warm-up: ~6us of dummy transposes releases the HAM clock
            # gate (1.2 -> 2.4 GHz) just as the real transposes start, and
            # keeps it released (gaps stay under the 3.4us idle window)
            wt = psum_ring.tile([P, P], bf16, tag="pg", name="warm")
            for _ in range(64):
                nc.tensor.transpose(wt[:], ident[:], ident[:])

            def prep_sq(s):
                # row sum-of-squares (DVE, bf16 2x)
                ssl = slice(s * MT, (s + 1) * MT)
                scr = scratch.tile([P, RPC], bf16, tag="scr", name=f"scr{s}")
                nc.vector.tensor_mul(scr[:], ld[s][:], ld[s][:])
                # bf16 row-norm^2 costs 0.4% relative error on |x| -- far
                # inside the 2e-2 loss tolerance (z is bf16 anyway)
                with nc.allow_low_precision(reason="bf16 norms, tol 2e-2"):
                    nc.vector.tensor_reduce(
                        ss[:, ssl].rearrange("p (t o) -> p t o", o=1),
                        scr[:].rearrange("p (t d) -> p t d", d=D),
                        axis=mybir.AxisListType.X, op=OP.add,
                    )

            def norm_inv(ssl):
                nc.scalar.activation(lns[:, ssl], ss[:, ssl], AF.Ln)
                nc.scalar.activation(inv[:, ssl], lns[:, ssl], AF.Exp, scale=-0.5)

            def prep_norm(s):
                # normalized bf16 rows: per-tile tensor_scalar (bf16 4x
                # mode, f32 per-partition scalar)
                for t in range(MT):
                    tsl = slice(t * D, (t + 1) * D)
                    c = s * MT + t
                    nc.vector.tensor_scalar_mul(
                        zn[s][:, tsl], ld[s][:, tsl], inv[:, c:c + 1]
                    )

            def prep(s):
                prep_sq(s)
                norm_inv(slice(s * MT, (s + 1) * MT))
                prep_norm(s)

            pt_tiles = {}

            def tp(s, half=None):
                # PE transposes for zT[s] (half=0/1 emits 4 of the 8)
                if s not in pt_tiles:
                    pt_tiles[s] = psum_ring.tile([P, RPC], bf16, tag="pg",
                                                 name=f"pt{s}")
                pt = pt_tiles[s]
                rng = range(MT) if half is None else range(4 * half, 4 * half + 4)
                for t in rng:
                    tsl = slice(t * D, (t + 1) * D)
                    nc.tensor.transpose(pt[:, tsl], zn[s][:, tsl], ident[:])

            def cp(s):
                # PSUM->SBUF copy for zT[s]
                nc.vector.tensor_copy(zT[s][:], pt_tiles[s][:])

            def tpcp(s):
                tp(s)
                cp(s)

            def positives():
                # pos[p,t] = zn0[row 8p+t] . zn4[row 8p+t] (cores 0-3 valid)
                pscr = scratch.tile([P, RPC], f32, tag="scr", name="pscr")
                nc.vector.tensor_mul(pscr[:], zn[0][:], zn[4][:])
                nc.vector.tensor_reduce(
                    outb[:, OUT_POS:OUT_POS + MT].rearrange("p (t o) -> p t o", o=1),
                    pscr[:].rearrange("p (t d) -> p t d", d=D),
                    axis=mybir.AxisListType.X, op=OP.add,
                )

            # ---- main loop: two passes over m ----
            def chunk_moving(m, c):
                if c < 8:
                    k, half = divmod(c, 2)
                    return zT[k][:, half * 512:(half + 1) * 512]
                off = 512 * (m // 4)
                return zT[4][:, off:off + 512]

            def cs_slot(m, c):
                # -> (slot, first_m, last_m) or None if no colsum needed
                if c < 2:
                    return None                      # diag block
                if c < 6:
                    return (c - 2, 0, 7)             # pass-1 chains
                if c < 8:
                    return (c - 6, 0, 7)             # pass-2: b3c0, b3c1
                return (2, 0, 3) if m < 4 else (3, 4, 7)

            def emit_sim(m, j):
                pg = psum_ring.tile([P, NBATCH * 512], f32, tag="pg",
                                    name=f"pg{m}_{j}")
                stat = zT[0][:, m * D:(m + 1) * D]
                for i in range(NBATCH):
                    c = j * NBATCH + i
                    nc.tensor.matmul(
                        pg[:, i * 512:(i + 1) * 512], stat, chunk_moving(m, c),
                        start=True, stop=True,
                    )
                return pg

            def emit_exp(m, j, pg, ebuf, eoff):
                nc.scalar.activation(
                    ebuf[:, eoff:eoff + NBATCH * 512], pg[:], AF.Exp,
                    scale=INV_T,
                    accum_out=outb[:, OUT_ES + 3 * m + j:OUT_ES + 3 * m + j + 1],
                )

            def emit_cs(m, j, cst, ebuf, eoff):
                for i in range(NBATCH):
                    c = j * NBATCH + i
                    slot = cs_slot(m, c)
                    if slot is None:
                        continue
                    s, m0, m1 = slot
                    bank, band = SLOT_POS[s]
                    nc.tensor.matmul(
                        cst[band:band + 32, 512 * bank:512 * (bank + 1)],
                        ones32[:, 0:32],
                        ebuf[:, eoff + i * 512:eoff + (i + 1) * 512],
                        start=(m == m0), stop=(m == m1),
                        skip_group_check=True,
                    )

            # Phase schedule (software pipelining):
            #   prep 0-2 -> j0 batches (slots 0,1) -> prep/transpose 2-4
            #   under the exp stream -> j1 batches (slots 1,2) -> pass-2
            #   batches (slots 3,4).  sim(k+1) is emitted before cs(k) so
            #   the PE never waits for ACT's exp of batch k.
            # DVE runs [sq0 red0 ts0 sq1 red1 ts1 cp0 cp1 ...] so the zT
            # copies (which wait on PE transposes) never block the next
            # slot's norm chain
            prep(0)
            prep(1)
            tp(0)
            tp(1)
            cp(0)
            cp(1)
            prep(2)

            prev = None

            def run_phase(j, cst, hooks=None):
                nonlocal prev
                for m in range(MT):
                    ebuf_m = expb.tile([P, NBATCH * 512], bf16,
                                       tag="eb", name=f"eb{j}_{m}")
                    pg = emit_sim(m, j)
                    if prev is not None:
                        emit_cs(*prev)
                    emit_exp(m, j, pg, ebuf_m, 0)
                    prev = (m, j, cst, ebuf_m, 0)
                    if hooks and m in hooks:
                        hooks[m]()

            def drain(cst, p):
                # drain only the written 32-row bands (bank0: 3, bank1: 1)
                lo = OUT_CS + p * 1024
                nc.vector.tensor_copy(outb[0:96, lo:lo + 512], cst[0:96, 0:512])
                nc.vector.tensor_copy(
                    outb[0:32, lo + 512:lo + 1024], cst[0:32, 512:1024]
                )

            def dma_cs(p):
                # band-gather the 4 written cs slots of pass p (8KB total)
                lo = OUT_CS + p * 1024
                b0 = outb[0:96, lo:lo + 512].rearrange(
                    "(a b) n -> a b n", b=32)[:, 0]
                nc.sync.dma_start(out=cs_d.ap()[4 * p:4 * p + 3], in_=b0)
                nc.sync.dma_start(out=cs_d.ap()[4 * p + 3:4 * p + 4],
                                  in_=outb[0:1, lo + 512:lo + 1024])

            def ln34():
                # slots 3/4 norms: one shared Ln/Exp pair slotted between
                # early j0 exps (squares were emitted in the prologue), and
                # the normalize multiplies on the otherwise-idle DVE; tp2
                # fills the PE slack while ACT runs exp(m0,j0)
                norm_inv(slice(3 * MT, 5 * MT))
                prep_norm(3)
                prep_norm(4)

            prep_sq(3)
            prep_sq(4)
            cst1 = psum_cs.tile([P, 2 * 512], f32, tag="cs", name="cs1")
            run_phase(0, cst1, hooks={
                0: ln34,
                2: lambda: tp(2, 0),
                3: lambda: (tp(2, 1), cp(2)),
                4: lambda: tp(3, 0),
                5: lambda: (tp(3, 1), cp(3)),
                6: lambda: tp(4, 0),
                7: lambda: (tp(4, 1), cp(4), positives()),
            })
            run_phase(1, cst1)
            emit_cs(*prev)
            prev = None
            drain(cst1, 0)
            dma_cs(0)
            cst2 = psum_cs.tile([P, 2 * 512], f32, tag="cs", name="cs2")
            run_phase(2, cst2)
            emit_cs(*prev)
            drain(cst2, 1)
            dma_cs(1)

            nc.sync.dma_start(out=out_d.ap(), in_=outb[:, 0:32])

    if hoist:
        _hoist_excess_waits(nc)
    return nc


def _get_nc(hoist: bool = True) -> bass.Bass:
    # hoist=False for CoreSim (the wait-carrier EventSemaphores it inserts
    # have no sem updates, which CoreSim's event loop rejects); True for HW.
    global _NC
    if _NC is None or getattr(_NC, "_bass_hoisted", None) != hoist:
        _NC = _build_nc(hoist)
        _NC._bass_hoisted = hoist
    return _NC


def core_gather(reps: np.ndarray, c: int) -> np.ndarray:
    """Host-side shard: 5 row groups for core c (slot 4 rotated on c>=4)."""
    grp = lambda g: reps[(g % NG) * RPC:((g % NG) + 1) * RPC]
    slots = [grp(c + k) for k in range(4)]
    if c < 4:
        slots.append(grp(c + 4))
    else:
        # roll by -4 rows: swaps the mod-8 row classes {0-3} <-> {4-7}
        # (a slot row r sits at (partition r//8, tile r%8) on chip, and the
        # gap-4 block is split by tile class, so this makes the two cores
        # of a pair cover complementary class-quadrants)
        slots.append(np.roll(grp(c + 4), -4, axis=0))
    return np.ascontiguousarray(np.concatenate(slots, axis=0))


# zT column l (transpose-tile t, entry q: l = t*128+q) holds slot row 8q+t
_PI = (8 * (np.arange(RPC) % P) + np.arange(RPC) // P).astype(np.int64)


def assemble(outs) -> float:
    """Host-side unshard: sum partials -> denom -> loss (float64)."""
    S = np.zeros(NR, dtype=np.float64)
    pos = np.zeros(B, dtype=np.float64)
    for c, (o1, ocs) in enumerate(outs):
        o1 = np.asarray(o1, dtype=np.float64)
        ocs = np.asarray(ocs, dtype=np.float64)
        # row-sums: esums[p, 3m+j] for local row 8p+m of group c
        es = o1[:, OUT_ES:OUT_ES + 24].reshape(P, MT, NBATCH).sum(axis=2)
        S[c * RPC:(c + 1) * RPC] += es.reshape(RPC)
        # col-sums: ocs row 4p+s = chain (pass p, slot s); zT col l -> row
        # PI[l].  pass 0: b1c0 b1c1 b2c0 b2c1; pass 1: b3c0 b3c1 half0 half1
        for p in range(2):
            for s in range(4):
                v = ocs[4 * p + s]
                ch = 2 + 4 * p + s               # global chunk id 2..9
                if ch < 8:
                    k, half = divmod(ch, 2)      # block k=1..3, col half
                    g = (c + k) % NG
                    S[g * RPC + _PI[half * 512:(half + 1) * 512]] += v
                else:
                    rho = _PI[(ch - 8) * 512:(ch - 7) * 512]  # slot4 rows
                    g = (c + 4) % NG
                    glob = rho if c < 4 else (rho + 4) % RPC
                    S[g * RPC + glob] += v
        if c < 4:
            pv = o1[:, OUT_POS:OUT_POS + MT]     # pos[p, t] -> row 8p+t
            pos[c * RPC:(c + 1) * RPC] = pv.reshape(RPC)
    denom = S - E2
    loss = (np.log(denom).sum() - 2.0 * INV_T * pos.sum()) / NR
    return float(np.float32(loss))


def kernel(emb_i: np.ndarray, emb_j: np.ndarray) -> np.ndarray:
    global _LAST_RESULT
    import ml_dtypes

    reps = np.ascontiguousarray(
        np.concatenate(
            [np.asarray(emb_i, np.float32), np.asarray(emb_j, np.float32)], axis=0
        )
    )
    assert reps.shape == (NR, D)

    in_maps = [
        {"gather": core_gather(reps, c).astype(ml_dtypes.bfloat16)}
        for c in range(N_CORES)
    ]

    kw = {}
    if TRACE:
        import os
        import tempfile

        kw["tmpdir"] = tempfile.mkdtemp(prefix="trace_", dir=os.getcwd())
    res = run_bass_kernel_spmd(
        _get_nc(), in_maps, list(range(N_CORES)), trace=TRACE, **kw
    )
    _LAST_RESULT = res

    return np.asarray(assemble([(r["out"], r["ocs"]) for r in res.results]))


# revision 43
# speedup vs baseline: 1.0668x; 1.0490x over previous
"""NT-Xent contrastive loss on 8 Trainium2 NeuronCores — symmetric-block v2.

Reference computation (B=4096, D=128, T=0.5):
    z = row-normalize(concat(emb_i, emb_j))           # [8192, 128]
    sim = z @ z.T                                     # [8192, 8192]
    S_r = sum_l exp(sim[r,l]/T),  denom_r = S_r - exp(sim[r,r]/T)
    pos_r = sim[r, r+-B]
    loss = mean_r ( log(denom_r) - pos_r/T )

sim is symmetric, so each [1024,1024] block (i,j) of exp(sim/T) only needs
to be exponentiated once: its row-sums feed rows of group i and its
column-sums feed rows of group j.  The 8x8 block grid has 36 unique blocks
(8 diagonal + 28 off-diagonal); each core computes exactly 4.5 of them:

    core c: blocks (c, c+k mod 8) for k=0..3  (diag + 3 off-diag)
            + half of the gap-4 pair {c, (c+4)%8}:
              cores 0-3 take the two diagonal quadrants of (c, c+4),
              cores 4-7 take the two anti-diagonal quadrants, expressed
              uniformly by feeding them slot 4 rotated by 512 rows.

This HALVES the ScalarE exp work vs the v1 kernel (exp throughput is
1 elem/lane/cycle and was the bottleneck).  Column-sums of exp blocks are
produced on the PE with a ones[128,32] stationary into quarter-partition
PSUM accumulators.  Each core returns raw partial sums (row-sums, col-sums,
positives); the host assembles S_r, subtracts the exp(sim_rr/T)=e^2
diagonal, and takes log/mean in float64.

Per-core inputs: `gather` [5*1024, 128] f32 = row groups
    [G(c), G(c+1), G(c+2), G(c+3), H]  (mod 8), where
    H = G(c+4) for c<4, and H = roll(G(c-4), -4 rows) for c>=4.
The SPMD program is identical on every core; only the data differs.

SBUF row layout: slot row r sits at (partition, tile) = (r // 8, r % 8),
so each slot's DMA is one fully contiguous 512KB DRAM stream (the
(p, t) = (r % 128, r // 128) layout read 512B-scattered at ~50GB/s and
made the prologue the bottleneck).  zT column t*128+q therefore holds row
8q+t; the host maps column positions back to rows.  "Half" of a group for
the gap-4 split is the mod-8 row class {0-3} vs {4-7}, which is why the
c>=4 rotation is roll(-4): it swaps the classes so the two cores of a
gap-4 pair cover complementary class-quadrants of the block.

Engine plan per core (predicted busy): ACT ~45us (24 Exp ACTIVATEs of
N=1536 from PSUM + accum row-sums), PE ~30us (72 sim matmuls N=512 bf16 —
one stationary per m-tile serves all 9 — 56 colsum matmuls, 40 bf16
transposes), DVE ~25us (square/reduce/normalize per slot + PSUM->SBUF
copies), DMA 2.7MB.  Norms use exp(-0.5*ln(s)) to stay in the single
natural_log_exp activation-table set (no 2.7us table swaps).
"""

import math

import numpy as np

import concourse.bass as bass
import concourse.mybir as mybir
import concourse.tile as tile
from concourse import masks
from concourse.bass_utils import run_bass_kernel_spmd

B = 4096
D = 128
NR = 2 * B               # 8192 rows of reps / sim
N_CORES = 8
RPC = NR // N_CORES      # 1024 rows per group
P = 128                  # partitions
NG = 8                   # global row groups of 1024
NS = 5                   # gather slots per core
MT = RPC // P            # 8 row tiles per group
TEMPERATURE = 0.5
INV_T = 1.0 / TEMPERATURE          # 2.0
E2 = math.exp(1.0 / TEMPERATURE)   # exp(sim_rr / T), sim_rr == 1

# main loop geometry: per m-tile, 9 chunks of 512 sim columns
#   chunk 0..1 -> block k=0 (diag), 2..3 -> k=1, 4..5 -> k=2, 6..7 -> k=3,
#   chunk 8    -> half-block (slot4 cols 0:512 for m<4, 512:1024 for m>=4)
# Two passes over m (AP base partitions are limited to {0,32,64}, so only
# 4 colsum chains fit the 2 spare PSUM banks at a time):
#   pass 1: batches j=0 (chunks 0,1,2) and j=1 (chunks 3,4,5) per m;
#           colsum chains b1c0,b1c1,b2c0,b2c1 -> drain to SBUF
#   pass 2: batch j=2 (chunks 6,7,8) per m;
#           colsum chains b3c0,b3c1,half(m<4),half(m>=4)
NCHUNK = 9
NBATCH = 3               # chunks per ACTIVATE batch (N=1536)
# cs slot -> (psum bank, partition band)
SLOT_POS = [(0, 0), (0, 32), (0, 64), (1, 0)]

# output packing: [128, 24 esums | 8 pos | 2*1024 colsum drains] f32
OUT_ES = 0               # esums[p, 3*m+j]
OUT_POS = 24             # pos[p, t]
OUT_CS = 32              # per pass: slot s at (bank,band): col 512*bank+k
OUT_W = 32 + 2 * 1024

_NC = None
TRACE = False            # test.py flips this for profiled runs
_LAST_RESULT = None      # test.py reads exec_time_ns / trace from here

f32 = mybir.dt.float32
bf16 = mybir.dt.bfloat16
AF = mybir.ActivationFunctionType
OP = mybir.AluOpType


def _patched_clear_and_free_semaphores(self, sems):
    """Replacement for Bass.clear_and_free_semaphores: the stock version
    emits a raw-ISA EVENT_SEMAPHORE_RANGE_CLEAR that this toolchain's walrus
    rejects ("ISA wrong length").  Emit BIR-native per-sem `wr-imm 0`
    updates on gpsimd NOPs instead — same semantics (sems reset between
    NEFF executions), supported lowering."""
    if not sems:
        return
    sem_nums = [s.num if hasattr(s, "num") else s for s in sems]
    for n in sem_nums:
        inst = self.gpsimd.nop()
        upd = mybir.SyncUpdate(
            sync_type="semaphore",
            id=n,
            update_mode="sem-wr-imm",
            update_value=0,
            ant_name=f"semclr{n}",
        )
        si = inst.ins.sync_info
        if si is None:
            inst.ins.sync_info = mybir.SyncInfo(on_wait=[], on_update=[upd])
        else:
            si.on_update.append(upd)
    self._state.prepend_free_semaphores(sem_nums)
    for poison_set in self._tile_sem_poison_stack:
        poison_set.update(sem_nums)


def _hoist_excess_waits(nc):
    """This toolchain's walrus (CoreV3GenImpl) allows only ONE sync-wait on
    most compute instruction structs; Tile sometimes attaches two.  Hoist
    all-but-one wait onto same-engine EventSemaphore carriers (2 wait slots
    each) inserted immediately before the instruction — same-engine program
    order makes this semantically identical."""
    n = 0
    for f in nc.m.functions:
        for blk in f.blocks:
            out = []
            for inst in blk.instructions:
                si = inst.sync_info
                tn = type(inst).__name__
                if (
                    si is not None
                    and len(si.on_wait) > 1
                    and tn != "InstEventSemaphore"
                ):
                    waits = list(si.on_wait)
                    keep, extra = waits[-1:], waits[:-1]
                    while extra:
                        grp, extra = extra[:2], extra[2:]
                        es = mybir.InstEventSemaphore(
                            name=f"wcarrier_{n}", ins=[], outs=[]
                        )
                        n += 1
                        es.engine = inst.engine
                        es.sync_info = mybir.SyncInfo(on_wait=list(grp), on_update=[])
                        out.append(es)
                    inst.sync_info = mybir.SyncInfo(
                        on_wait=keep, on_update=list(si.on_update)
                    )
                out.append(inst)
            blk.instructions[:] = out


def _build_nc(hoist: bool = True) -> bass.Bass:
    nc = bass.Bass("TRN2", target_bir_lowering=False, debug=False)
    import types as _types

    nc.clear_and_free_semaphores = _types.MethodType(
        _patched_clear_and_free_semaphores, nc
    )

    gather = nc.dram_tensor("gather", [NS * RPC, D], bf16, kind="ExternalInput")
    out_d = nc.dram_tensor("out", [P, 32], f32, kind="ExternalOutput")
    cs_d = nc.dram_tensor("ocs", [8, 512], f32, kind="ExternalOutput")

    with tile.TileContext(nc) as tc:
        with (
            tc.tile_pool(name="singles", bufs=1) as singles,
            tc.tile_pool(name="scratch", bufs=2) as scratch,
            tc.tile_pool(name="expb", bufs=2) as expb,
            tc.tile_pool(name="psum_ring", bufs=2, space="PSUM") as psum_ring,
            # prologue transpose tiles and the colsum accumulators share one
            # 2-bank pool (their lifetimes don't overlap; Tile serializes)
            tc.tile_pool(name="psum_cs", bufs=1, space="PSUM") as psum_cs,
        ):
            ident = singles.tile([P, P], bf16, tag="ident")
            masks.make_identity(nc, ident[:])
            ones32 = singles.tile([P, 32], bf16, tag="ones32")
            nc.vector.memset(ones32[:], 1.0)

            # persistent SBUF buffers
            ld = [
                singles.tile([P, RPC], bf16, name=f"ld{s}", tag=f"ld{s}")
                for s in range(NS)
            ]
            zn = [
                singles.tile([P, RPC], bf16, name=f"zn{s}", tag=f"zn{s}")
                for s in range(NS)
            ]
            zT = [
                singles.tile([P, RPC], bf16, name=f"zT{s}", tag=f"zT{s}")
                for s in range(NS)
            ]
            ss = singles.tile([P, NS * MT], bf16, tag="ss")
            lns = singles.tile([P, NS * MT], f32, tag="lns")
            inv = singles.tile([P, NS * MT], f32, tag="inv")
            outb = singles.tile([P, OUT_W], f32, tag="outb")

            # slot row r at (partition, tile) = (r//8, r%8): contiguous DMA;
            # two half-slot DMAs each so transfers overlap across DMA
            # engines.  Slots 0-1 issue from the Sync queue; slots 2-4 from
            # the Scalar queue (idle until the first Ln anyway) so issue
            # serialization doesn't delay the early slots.
            gv = gather.ap().rearrange("(s p h t) d -> s p h t d",
                                       s=NS, h=2, t=MT // 2)

            # force the ACT activation-table load ahead of the Scalar-queue
            # DMA issues so the first real Ln isn't pushed behind them
            nc.scalar.activation(lns[:, 0:1], ones32[:, 0:1], AF.Ln)

            for s in range(NS):
                ldv = ld[s][:].rearrange("p (h t d) -> p h t d", h=2, d=D)
                eng = nc.sync if s < 2 else nc.scalar
                eng.dma_start(out=ldv[:, 0], in_=gv[s][:, 0])
                eng.dma_start(out=ldv[:, 1], in_=gv[s][:, 1])

            # PE warm-up: ~6us of dummy transposes releases the HAM clock
            # gate (1.2 -> 2.4 GHz) just as the real transposes start, and
            # keeps it released (gaps stay under the 3.4us idle window)
            wt = psum_ring.tile([P, P], bf16, tag="pg", name="warm")
            for _ in range(64):
                nc.tensor.transpose(wt[:], ident[:], ident[:])

            def prep_sq(s):
                # row sum-of-squares (DVE, bf16 2x)
                ssl = slice(s * MT, (s + 1) * MT)
                scr = scratch.tile([P, RPC], bf16, tag="scr", name=f"scr{s}")
                nc.vector.tensor_mul(scr[:], ld[s][:], ld[s][:])
                # bf16 row-norm^2 costs 0.4% relative error on |x| -- far
                # inside the 2e-2 loss tolerance (z is bf16 anyway)
                with nc.allow_low_precision(reason="bf16 norms, tol 2e-2"):
                    nc.vector.tensor_reduce(
                        ss[:, ssl].rearrange("p (t o) -> p t o", o=1),
                        scr[:].rearrange("p (t d) -> p t d", d=D),
                        axis=mybir.AxisListType.X, op=OP.add,
                    )

            def norm_inv(ssl):
                nc.scalar.activation(lns[:, ssl], ss[:, ssl], AF.Ln)
                nc.scalar.activation(inv[:, ssl], lns[:, ssl], AF.Exp, scale=-0.5)

            def prep_norm(s):
                # normalized bf16 rows: per-tile tensor_scalar (bf16 4x
                # mode, f32 per-partition scalar)
                for t in range(MT):
                    tsl = slice(t * D, (t + 1) * D)
                    c = s * MT + t
                    nc.vector.tensor_scalar_mul(
                        zn[s][:, tsl], ld[s][:, tsl], inv[:, c:c + 1]
                    )

            def prep(s):
                prep_sq(s)
                norm_inv(slice(s * MT, (s + 1) * MT))
                prep_norm(s)

            pt_tiles = {}

            def tp(s, half=None):
                # PE transposes for zT[s] (half=0/1 emits 4 of the 8)
                if s not in pt_tiles:
                    pt_tiles[s] = psum_ring.tile([P, RPC], bf16, tag="pg",
                                                 name=f"pt{s}")
                pt = pt_tiles[s]
                rng = range(MT) if half is None else range(4 * half, 4 * half + 4)
                for t in rng:
                    tsl = slice(t * D, (t + 1) * D)
                    nc.tensor.transpose(pt[:, tsl], zn[s][:, tsl], ident[:])

            def cp(s):
                # PSUM->SBUF copy for zT[s]
                nc.vector.tensor_copy(zT[s][:], pt_tiles[s][:])

            def tp_xbar(s):
                # zT[s] via DMA XBAR transposes: no PE/DVE/PSUM cost, just
                # Sync-queue issues -- used for the slots with time slack
                for t in range(MT):
                    tsl = slice(t * D, (t + 1) * D)
                    nc.sync.dma_start(out=zT[s][:, tsl], in_=zn[s][:, tsl],
                                      transpose=True)

            def positives():
                # pos[p,t] = zn0[row 8p+t] . zn4[row 8p+t] (cores 0-3 valid)
                pscr = scratch.tile([P, RPC], f32, tag="scr", name="pscr")
                nc.vector.tensor_mul(pscr[:], zn[0][:], zn[4][:])
                nc.vector.tensor_reduce(
                    outb[:, OUT_POS:OUT_POS + MT].rearrange("p (t o) -> p t o", o=1),
                    pscr[:].rearrange("p (t d) -> p t d", d=D),
                    axis=mybir.AxisListType.X, op=OP.add,
                )

            # ---- main loop: two passes over m ----
            def chunk_moving(m, c):
                if c < 8:
                    k, half = divmod(c, 2)
                    return zT[k][:, half * 512:(half + 1) * 512]
                off = 512 * (m // 4)
                return zT[4][:, off:off + 512]

            def cs_slot(m, c):
                # -> (slot, first_m, last_m) or None if no colsum needed
                if c < 2:
                    return None                      # diag block
                if c < 6:
                    return (c - 2, 0, 7)             # pass-1 chains
                if c < 8:
                    return (c - 6, 0, 7)             # pass-2: b3c0, b3c1
                return (2, 0, 3) if m < 4 else (3, 4, 7)

            def emit_sim(m, j):
                pg = psum_ring.tile([P, NBATCH * 512], f32, tag="pg",
                                    name=f"pg{m}_{j}")
                stat = zT[0][:, m * D:(m + 1) * D]
                for i in range(NBATCH):
                    c = j * NBATCH + i
                    nc.tensor.matmul(
                        pg[:, i * 512:(i + 1) * 512], stat, chunk_moving(m, c),
                        start=True, stop=True,
                    )
                return pg

            def emit_exp(m, j, pg, ebuf, eoff):
                nc.scalar.activation(
                    ebuf[:, eoff:eoff + NBATCH * 512], pg[:], AF.Exp,
                    scale=INV_T,
                    accum_out=outb[:, OUT_ES + 3 * m + j:OUT_ES + 3 * m + j + 1],
                )

            def emit_cs(m, j, cst, ebuf, eoff):
                for i in range(NBATCH):
                    c = j * NBATCH + i
                    slot = cs_slot(m, c)
                    if slot is None:
                        continue
                    s, m0, m1 = slot
                    bank, band = SLOT_POS[s]
                    nc.tensor.matmul(
                        cst[band:band + 32, 512 * bank:512 * (bank + 1)],
                        ones32[:, 0:32],
                        ebuf[:, eoff + i * 512:eoff + (i + 1) * 512],
                        start=(m == m0), stop=(m == m1),
                        skip_group_check=True,
                    )

            # Phase schedule (software pipelining):
            #   prep 0-2 -> j0 batches (slots 0,1) -> prep/transpose 2-4
            #   under the exp stream -> j1 batches (slots 1,2) -> pass-2
            #   batches (slots 3,4).  sim(k+1) is emitted before cs(k) so
            #   the PE never waits for ACT's exp of batch k.
            # DVE runs [sq0 red0 ts0 sq1 red1 ts1 cp0 cp1 ...] so the zT
            # copies (which wait on PE transposes) never block the next
            # slot's norm chain
            prep(0)
            prep(1)
            tp(0)
            tp(1)
            cp(0)
            cp(1)
            prep(2)
            tp_xbar(2)

            prev = None

            def run_phase(j, cst, hooks=None):
                nonlocal prev
                for m in range(MT):
                    ebuf_m = expb.tile([P, NBATCH * 512], bf16,
                                       tag="eb", name=f"eb{j}_{m}")
                    pg = emit_sim(m, j)
                    if prev is not None:
                        emit_cs(*prev)
                    emit_exp(m, j, pg, ebuf_m, 0)
                    prev = (m, j, cst, ebuf_m, 0)
                    if hooks and m in hooks:
                        hooks[m]()

            def drain(cst, p):
                # drain only the written 32-row bands (bank0: 3, bank1: 1)
                lo = OUT_CS + p * 1024
                nc.vector.tensor_copy(outb[0:96, lo:lo + 512], cst[0:96, 0:512])
                nc.vector.tensor_copy(
                    outb[0:32, lo + 512:lo + 1024], cst[0:32, 512:1024]
                )

            def dma_cs(p):
                # band-gather the 4 written cs slots of pass p (8KB total)
                lo = OUT_CS + p * 1024
                b0 = outb[0:96, lo:lo + 512].rearrange(
                    "(a b) n -> a b n", b=32)[:, 0]
                nc.sync.dma_start(out=cs_d.ap()[4 * p:4 * p + 3], in_=b0)
                nc.sync.dma_start(out=cs_d.ap()[4 * p + 3:4 * p + 4],
                                  in_=outb[0:1, lo + 512:lo + 1024])

            def ln34():
                # slots 3/4 norms: one shared Ln/Exp pair slotted between
                # early j0 exps (squares were emitted in the prologue), and
                # the normalize multiplies on the otherwise-idle DVE; tp2
                # fills the PE slack while ACT runs exp(m0,j0)
                norm_inv(slice(3 * MT, 5 * MT))
                prep_norm(3)
                prep_norm(4)

            prep_sq(3)
            prep_sq(4)
            cst1 = psum_cs.tile([P, 2 * 512], f32, tag="cs", name="cs1")
            run_phase(0, cst1, hooks={
                0: ln34,
                1: lambda: (tp_xbar(3), tp_xbar(4)),
                7: positives,
            })
            run_phase(1, cst1)
            emit_cs(*prev)
            prev = None
            drain(cst1, 0)
            dma_cs(0)
            cst2 = psum_cs.tile([P, 2 * 512], f32, tag="cs", name="cs2")
            run_phase(2, cst2)
            emit_cs(*prev)
            drain(cst2, 1)
            dma_cs(1)

            nc.sync.dma_start(out=out_d.ap(), in_=outb[:, 0:32])

    if hoist:
        _hoist_excess_waits(nc)
    return nc


def _get_nc(hoist: bool = True) -> bass.Bass:
    # hoist=False for CoreSim (the wait-carrier EventSemaphores it inserts
    # have no sem updates, which CoreSim's event loop rejects); True for HW.
    global _NC
    if _NC is None or getattr(_NC, "_bass_hoisted", None) != hoist:
        _NC = _build_nc(hoist)
        _NC._bass_hoisted = hoist
    return _NC


def core_gather(reps: np.ndarray, c: int) -> np.ndarray:
    """Host-side shard: 5 row groups for core c (slot 4 rotated on c>=4)."""
    grp = lambda g: reps[(g % NG) * RPC:((g % NG) + 1) * RPC]
    slots = [grp(c + k) for k in range(4)]
    if c < 4:
        slots.append(grp(c + 4))
    else:
        # roll by -4 rows: swaps the mod-8 row classes {0-3} <-> {4-7}
        # (a slot row r sits at (partition r//8, tile r%8) on chip, and the
        # gap-4 block is split by tile class, so this makes the two cores
        # of a pair cover complementary class-quadrants)
        slots.append(np.roll(grp(c + 4), -4, axis=0))
    return np.ascontiguousarray(np.concatenate(slots, axis=0))


# zT column l (transpose-tile t, entry q: l = t*128+q) holds slot row 8q+t
_PI = (8 * (np.arange(RPC) % P) + np.arange(RPC) // P).astype(np.int64)


def assemble(outs) -> float:
    """Host-side unshard: sum partials -> denom -> loss (float64)."""
    S = np.zeros(NR, dtype=np.float64)
    pos = np.zeros(B, dtype=np.float64)
    for c, (o1, ocs) in enumerate(outs):
        o1 = np.asarray(o1, dtype=np.float64)
        ocs = np.asarray(ocs, dtype=np.float64)
        # row-sums: esums[p, 3m+j] for local row 8p+m of group c
        es = o1[:, OUT_ES:OUT_ES + 24].reshape(P, MT, NBATCH).sum(axis=2)
        S[c * RPC:(c + 1) * RPC] += es.reshape(RPC)
        # col-sums: ocs row 4p+s = chain (pass p, slot s); zT col l -> row
        # PI[l].  pass 0: b1c0 b1c1 b2c0 b2c1; pass 1: b3c0 b3c1 half0 half1
        for p in range(2):
            for s in range(4):
                v = ocs[4 * p + s]
                ch = 2 + 4 * p + s               # global chunk id 2..9
                if ch < 8:
                    k, half = divmod(ch, 2)      # block k=1..3, col half
                    g = (c + k) % NG
                    S[g * RPC + _PI[half * 512:(half + 1) * 512]] += v
                else:
                    rho = _PI[(ch - 8) * 512:(ch - 7) * 512]  # slot4 rows
                    g = (c + 4) % NG
                    glob = rho if c < 4 else (rho + 4) % RPC
                    S[g * RPC + glob] += v
        if c < 4:
            pv = o1[:, OUT_POS:OUT_POS + MT]     # pos[p, t] -> row 8p+t
            pos[c * RPC:(c + 1) * RPC] = pv.reshape(RPC)
    denom = S - E2
    loss = (np.log(denom).sum() - 2.0 * INV_T * pos.sum()) / NR
    return float(np.float32(loss))


def kernel(emb_i: np.ndarray, emb_j: np.ndarray) -> np.ndarray:
    global _LAST_RESULT
    import ml_dtypes

    reps = np.ascontiguousarray(
        np.concatenate(
            [np.asarray(emb_i, np.float32), np.asarray(emb_j, np.float32)], axis=0
        )
    )
    assert reps.shape == (NR, D)

    in_maps = [
        {"gather": core_gather(reps, c).astype(ml_dtypes.bfloat16)}
        for c in range(N_CORES)
    ]

    kw = {}
    if TRACE:
        import os
        import tempfile

        kw["tmpdir"] = tempfile.mkdtemp(prefix="trace_", dir=os.getcwd())
    res = run_bass_kernel_spmd(
        _get_nc(), in_maps, list(range(N_CORES)), trace=TRACE, **kw
    )
    _LAST_RESULT = res

    return np.asarray(assemble([(r["out"], r["ocs"]) for r in res.results]))
